# revision 19
# baseline (speedup 1.0000x reference)
"""Trainium2 Bass kernel for nn_FeatureBuilder (pairwise RBF feature builder).

Strategy: data-parallel over the batch (graph) axis -- each of the 8
NeuronCores processes one padded graph [N=256 atoms]. Embedding tables /
RBF weights are replicated. Inside a core everything runs in an
[e(partition), pair(free)] layout.

Per unit of 4 atom-rows (1024 pairs):
  - the gaussian exponent arg  coeff*(d - c_k)^2  is expanded to
    coeff*d^2 - 2*coeff*c_k*d  (+ coeff*c_k^2 via the ScalarE bias) and
    built by a K=8 float32r matmul whose rows are exact 12-bit hi/lo
    (Dekker) splits of (d^2, d) x (coeff, -2*coeff*c_k) -- full fp32
    accuracy at the PE's 1-cycle/row float32r rate, landing directly in
    the [gaussian, pair] layout (no transposes, no broadcasts).
  - ONE ScalarE Exp (bias = coeff*c_k^2 per partition) -> g.
  - K=66 float32r matmul: rows 0-49 rbf_w/sqrt3 (stationary), rows
    50-65 a 16-row block of src-embedding rows (one slab DMA per 4
    units), selected by constant indicator rows preloaded in 4 rotating
    g tiles.
  - one DVE tensor_tensor adds the dst-embedding table during the
    mandatory PSUM->SBUF pass.
  - feats DMA'd in contiguous 1MB blocks (one DMA per 2 units).
"""

import math

import numpy as np

import concourse.bacc as bacc
import concourse.mybir as mybir
import concourse.tile as tile
from concourse.bass_utils import run_bass_kernel_spmd

# ---- problem constants (hardcoded per spec) ----
B = 8          # graphs == cores
N = 256        # padded atoms per graph (nmax)
P = 128        # partitions
NT = N // P    # i-tiles per graph
E = 128        # embed dim
G = 50         # gaussians
NELEM = 100
RADIUS = 12.0
SQRT3 = math.sqrt(3.0)

UNITS = N // 4          # 64 units of 4 atom-rows (1024 pairs each)
CHUNK_I = 64            # i-rows per sd chunk
UPC = 16                # units per chunk
NBLK = UNITS // 2       # feats DMA blocks (2 units each)

FP32 = mybir.dt.float32
FP32R = mybir.dt.float32r
AF = mybir.ActivationFunctionType
OP = mybir.AluOpType


def _split12(x):
    """Exact split of f32 array into hi (top 12 significand bits) + lo."""
    x = np.asarray(x, dtype=np.float32)
    hi = (x.view(np.uint32) & np.uint32(0xFFFFF000)).view(np.float32)
    lo = (x - hi).astype(np.float32)
    return hi, lo


def _rbf_consts():
    # match reference: float32 centers, coeff from f32 spacing
    centers = np.linspace(0.0, RADIUS, G, dtype=np.float32).astype(np.float64)
    coeff = -0.5 / float(np.float32(centers[1] - centers[0])) ** 2
    return centers, coeff


def _build():
    nc = bacc.Bacc("TRN2", target_bir_lowering=False, debug=False)

    # inputs (per-core shard + replicated tables)
    pos_pt = nc.dram_tensor("pos_pt", [NT, P, 3], FP32, kind="ExternalInput")
    pos_r = [
        nc.dram_tensor(f"pos_r{c}", [1, N], FP32, kind="ExternalInput")
        for c in range(3)
    ]
    anum_row = nc.dram_tensor("anum_row", [1, N], FP32, kind="ExternalInput")
    w_s = nc.dram_tensor("w_s", [G, E], FP32R, kind="ExternalInput")
    semb_s = nc.dram_tensor("semb_s", [NELEM, E], FP32, kind="ExternalInput")
    brow = nc.dram_tensor("brow", [1, E], FP32, kind="ExternalInput")
    demb_s = nc.dram_tensor("demb_s", [NELEM, E], FP32, kind="ExternalInput")
    qsplit = nc.dram_tensor("qsplit", [8, G], FP32R, kind="ExternalInput")
    qcol = nc.dram_tensor("qcol", [G, 1], FP32, kind="ExternalInput")
    iota_col = nc.dram_tensor("iota_col", [NELEM, 1], FP32, kind="ExternalInput")
    indv = nc.dram_tensor("indv", [64, 1024], FP32R, kind="ExternalInput")

    # outputs
    feats_o = nc.dram_tensor("feats", [NBLK, E, 2048], FP32, kind="ExternalOutput")
    dist_o = nc.dram_tensor("dist", [NT, P, N], FP32, kind="ExternalOutput")
    vh_o = nc.dram_tensor("vh", [3, NT, P, N], FP32, kind="ExternalOutput")

    with tile.TileContext(nc) as tc:
        with (
            tc.tile_pool(name="const", bufs=1) as cpool,
            tc.tile_pool(name="dwork", bufs=2) as dpool,
            tc.tile_pool(name="sd", bufs=1) as sdpool,
            tc.tile_pool(name="osb", bufs=3) as opool,
            tc.tile_pool(name="pf", bufs=2, space="PSUM") as pfpool,
            tc.tile_pool(name="pbc", bufs=2, space="PSUM") as pbcpool,
        ):
            # ---- load constants ----
            semb_sb = cpool.tile([NELEM, E], FP32)
            nc.sync.dma_start(semb_sb[:], semb_s[:])
            brow_sb = cpool.tile([1, E], FP32)
            nc.sync.dma_start(brow_sb[:], brow[:])
            demb_sb = cpool.tile([NELEM, E], FP32)
            nc.sync.dma_start(demb_sb[:], demb_s[:])
            # lives at partitions 96..103 so the K=8 exponent matmuls run in
            # PE row-group 3, concurrent with the K=66 feature matmuls
            qsplit_sb = cpool.tile([104, G], FP32R)
            nc.sync.dma_start(qsplit_sb[96:104, :], qsplit[:])
            qcol_sb = cpool.tile([G, 1], FP32)
            nc.sync.dma_start(qcol_sb[:], qcol[:])
            iota_sb = cpool.tile([NELEM, 1], FP32)
            nc.sync.dma_start(iota_sb[:], iota_col[:])
            anum_sb = cpool.tile([1, N], FP32)
            nc.sync.dma_start(anum_sb[:], anum_row[:])
            posr_sb = []
            for c in range(3):
                t_ = cpool.tile([1, N], FP32, tag=f"posr{c}")
                nc.sync.dma_start(t_[:], pos_r[c][:])
                posr_sb.append(t_)
            pt_sb = []
            for t in range(NT):
                t_ = cpool.tile([P, 3], FP32, tag=f"pt{t}")
                nc.sync.dma_start(t_[:], pos_pt[t])
                pt_sb.append(t_)

            ones100 = cpool.tile([1, NELEM], FP32)
            nc.gpsimd.memset(ones100[:], 1.0)
            ones128 = cpool.tile([1, P], FP32)
            nc.gpsimd.memset(ones128[:], 1.0)

            # ---- one-hot H^T [elem, atom] ----
            ps_an = pfpool.tile([NELEM, N], FP32, tag="pf")
            nc.tensor.matmul(ps_an[:], ones100[:], anum_sb[:])  # bcast anum rows
            Ht = cpool.tile([NELEM, N], FP32)
            nc.vector.tensor_scalar(Ht[:], ps_an[:], iota_sb[:], None, OP.is_equal)

            # src_pb[t][i_loc, e] = (src_emb[anum_i, e] + rbf_b[e]) / sqrt3
            src_pb = []
            for t in range(NT):
                ps_src = pfpool.tile([P, E], FP32, tag="pf")
                nc.tensor.matmul(
                    ps_src[:], Ht[:, t * P : (t + 1) * P], semb_sb[:],
                    start=True, stop=False,
                )
                nc.tensor.matmul(
                    ps_src[:], ones128[:], brow_sb[:], start=False, stop=True
                )
                t_ = cpool.tile([P, E], FP32, tag=f"srcpb{t}")
                nc.scalar.copy(t_[:], ps_src[:])
                src_pb.append(t_)

            # dstT4[e, 4*N] = dst_emb[anum_j, e]/sqrt3, repeated 4x along free
            ps_dst = pfpool.tile([E, N], FP32, tag="pf")
            nc.tensor.matmul(ps_dst[:], demb_sb[:], Ht[:])
            dstT4 = cpool.tile([E, 4 * N], FP32)
            for r in range(4):
                nc.scalar.copy(dstT4[:, r * N : (r + 1) * N], ps_dst[:])

            # ---- pos rows broadcast across partitions: pxb[p, j] = x_j ----
            pb_sb = []
            for c in range(3):
                ps_pb = pfpool.tile([P, N], FP32, tag="pf")
                nc.tensor.matmul(ps_pb[:], ones128[:], posr_sb[c][:])
                t_ = cpool.tile([P, N], FP32, tag=f"pb{c}")
                nc.scalar.copy(t_[:], ps_pb[:])
                pb_sb.append(t_)

            # ---- double-buffered lhsT for the feature matmul ----
            # rows 0..49 = rbf_w/sqrt3 (constant), rows 50..65 = a 16-row
            # block of src embedding rows (DMA'd once per 4 units)
            w66 = []
            for r in range(2):
                t_ = cpool.tile([G + 16, E], FP32R, tag=f"w66_{r}")
                nc.sync.dma_start(t_[0:G, :], w_s[:])
                w66.append(t_)

            # g tiles: rows 0..49 written by ACT each unit; rows 50..65 are
            # the constant indicator pattern for this unit's slab offset
            g66 = []
            for m in range(4):
                t_ = cpool.tile([G + 16, 4 * N], FP32R, tag=f"g66_{m}")
                nc.sync.dma_start(t_[G : G + 16, :], indv[16 * m : 16 * (m + 1), :])
                g66.append(t_)

            # ---- per i-tile: distances, vec_hat, then the unit loop ----
            for t in range(NT):
                dvec = []
                for c in range(3):
                    dc = dpool.tile([P, N], FP32, tag=f"d{c}")
                    # vec[i, j] = pos[j] - pos[i]
                    nc.vector.tensor_scalar(
                        dc[:], pb_sb[c][:], pt_sb[t][:, c : c + 1], None, OP.subtract
                    )
                    dvec.append(dc)
                s_t = dpool.tile([P, N], FP32, tag="s")
                nc.vector.tensor_mul(s_t[:], dvec[0][:], dvec[0][:])
                tmp = dpool.tile([P, N], FP32, tag="tmp")
                nc.vector.tensor_mul(tmp[:], dvec[1][:], dvec[1][:])
                nc.vector.tensor_add(s_t[:], s_t[:], tmp[:])
                tmp2 = dpool.tile([P, N], FP32, tag="tmp2")
                nc.vector.tensor_mul(tmp2[:], dvec[2][:], dvec[2][:])
                nc.vector.tensor_add(s_t[:], s_t[:], tmp2[:])

                d_t = dpool.tile([P, N], FP32, tag="d")
                nc.scalar.sqrt(d_t[:], s_t[:])
                nc.sync.dma_start(dist_o[t], d_t[:])

                clamp = dpool.tile([P, N], FP32, tag="clamp")
                nc.vector.tensor_scalar(clamp[:], d_t[:], 1e-12, None, OP.max)
                inv = dpool.tile([P, N], FP32, tag="inv")
                nc.vector.reciprocal(inv[:], clamp[:])
                for c in range(3):
                    vh_t = dpool.tile([P, N], FP32, tag=f"vh{c}")
                    nc.vector.tensor_mul(vh_t[:], dvec[c][:], inv[:])
                    nc.sync.dma_start(vh_o[c, t], vh_t[:])

                # exact 12-bit hi/lo splits of s (=d^2) and d
                s_hi = dpool.tile([P, N], FP32, tag="s_hi")
                nc.vector.tensor_scalar(
                    s_hi[:].bitcast(mybir.dt.int32), s_t[:].bitcast(mybir.dt.int32),
                    -4096, None, OP.bitwise_and,
                )
                s_lo = dpool.tile([P, N], FP32, tag="s_lo")
                nc.vector.tensor_sub(s_lo[:], s_t[:], s_hi[:])
                d_hi = dpool.tile([P, N], FP32, tag="d_hi")
                nc.vector.tensor_scalar(
                    d_hi[:].bitcast(mybir.dt.int32), d_t[:].bitcast(mybir.dt.int32),
                    -4096, None, OP.bitwise_and,
                )
                d_lo = dpool.tile([P, N], FP32, tag="d_lo")
                nc.vector.tensor_sub(d_lo[:], d_t[:], d_hi[:])

                for h in range(2):  # 2 chunks per i-tile
                    i0 = h * CHUNK_I  # local i offset of chunk
                    # sd8 rows 96..103: sh, sl, sh, sl, dh, dl, dh, dl
                    sd8 = sdpool.tile([104, CHUNK_I * N], FP32R, tag="sd")
                    for r, (eng, src_t) in enumerate(
                        (
                            (nc.sync, s_hi), (nc.sync, s_lo),
                            (nc.scalar, s_hi), (nc.scalar, s_lo),
                            (nc.sync, d_hi), (nc.sync, d_lo),
                            (nc.scalar, d_hi), (nc.scalar, d_lo),
                        )
                    ):
                        row = 96 + r
                        eng.dma_start(
                            sd8[row : row + 1, :],
                            src_t[i0 : i0 + CHUNK_I, :].bitcast(FP32R),
                        )

                    for v in range(UPC):  # 16 units per chunk
                        u = (t * 2 + h) * UPC + v
                        m = v % 4

                        if m == 0:
                            # 16-row src slab for the next 4 units
                            wbuf = w66[(u // 4) % 2]
                            nc.sync.dma_start(
                                wbuf[G : G + 16, :],
                                src_pb[t][
                                    i0 + 16 * (v // 4) : i0 + 16 * (v // 4) + 16, :
                                ].bitcast(FP32R),
                            )

                        ps_bc = pbcpool.tile([G, 4 * N], FP32, tag="pbc")
                        for q in range(2):
                            nc.tensor.matmul(
                                ps_bc[:, q * 512 : (q + 1) * 512],
                                qsplit_sb[96:104, :],
                                sd8[
                                    96:104,
                                    4 * v * N + q * 512 : 4 * v * N + (q + 1) * 512,
                                ],
                                tile_position=(96, 0),
                            )
                        gbuf = g66[m]
                        nc.scalar.activation(
                            gbuf[0:G, :], ps_bc[:], AF.Exp, bias=qcol_sb[:], scale=1.0
                        )
                        ps_f = pfpool.tile([E, 4 * N], FP32, tag="pf")
                        for q in range(2):
                            nc.tensor.matmul(
                                ps_f[:, q * 512 : (q + 1) * 512],
                                wbuf[:],
                                gbuf[:, q * 512 : (q + 1) * 512],
                            )
                        if u % 2 == 0:
                            o_sb = opool.tile([E, 2 * 4 * N], FP32, tag="osb")
                        nc.vector.tensor_add(
                            o_sb[:, (u % 2) * 1024 : (u % 2) * 1024 + 1024],
                            ps_f[:],
                            dstT4[:],
                        )
                        if u % 2 == 1:
                            nc.sync.dma_start(feats_o[u // 2], o_sb[:])

    nc.compile()
    return nc


_NC = None


def _get_nc():
    global _NC
    if _NC is None:
        _NC = _build()
    return _NC


def _make_in_maps(pos, natoms, atomic_numbers, src_emb, dst_emb, rbf_w, rbf_b):
    natoms = np.asarray(natoms).astype(np.int64)
    pos = np.asarray(pos, dtype=np.float32)
    anum = np.asarray(atomic_numbers).astype(np.int64)
    src = np.asarray(src_emb, dtype=np.float64)
    dst = np.asarray(dst_emb, dtype=np.float64)
    w = np.asarray(rbf_w, dtype=np.float64)
    bvec = np.asarray(rbf_b, dtype=np.float64)

    offs = np.concatenate([[0], np.cumsum(natoms)]).astype(np.int64)
    pp = np.zeros((B, N, 3), np.float32)
    pa = np.zeros((B, N), np.int64)
    for bi in range(B):
        na = int(natoms[bi])
        pp[bi, :na] = pos[offs[bi] : offs[bi] + na]
        pa[bi, :na] = anum[offs[bi] : offs[bi] + na]

    centers, coeff = _rbf_consts()
    ch, cl = _split12(np.full(G, coeff))
    q1h, q1l = _split12(-2.0 * coeff * centers)
    qsplit = np.stack([ch, ch, cl, cl, q1h, q1h, q1l, q1l], axis=0)
    qcol = (coeff * centers**2).astype(np.float32).reshape(G, 1)

    # indicator variants: variant m (unit u with u%4==m) activates slab rows
    # 4m..4m+3, one per atom-row of the unit
    indv = np.zeros((4, 16, 4 * N), np.float32)
    for m in range(4):
        for k in range(4):
            indv[m, 4 * m + k, k * N : (k + 1) * N] = 1.0
    indv = indv.reshape(64, 4 * N)

    shared = {
        "w_s": np.ascontiguousarray((w / SQRT3).astype(np.float32)),
        "semb_s": np.ascontiguousarray((src / SQRT3).astype(np.float32)),
        "brow": np.ascontiguousarray((bvec[None, :] / SQRT3).astype(np.float32)),
        "demb_s": np.ascontiguousarray((dst / SQRT3).astype(np.float32)),
        "qsplit": np.ascontiguousarray(qsplit.astype(np.float32)),
        "qcol": np.ascontiguousarray(qcol),
        "iota_col": np.arange(NELEM, dtype=np.float32).reshape(NELEM, 1),
        "indv": np.ascontiguousarray(indv),
    }
    in_maps = []
    for bi in range(B):
        m = dict(shared)
        m["pos_pt"] = np.ascontiguousarray(pp[bi].reshape(NT, P, 3))
        for c in range(3):
            m[f"pos_r{c}"] = np.ascontiguousarray(pp[bi][:, c].reshape(1, N))
        m["anum_row"] = np.ascontiguousarray(pa[bi].astype(np.float32).reshape(1, N))
        in_maps.append(m)
    return in_maps, natoms


def _assemble(results, natoms):
    # feats dev layout: [NBLK, E, 2048] with block b = atom rows 8b..8b+7
    feats = (
        np.stack([results[bi]["feats"] for bi in range(B)], axis=0)
        .reshape(B, NBLK, E, 8, N)
        .transpose(1, 3, 4, 0, 2)
        .reshape(N, N, B, E)
    )
    dist = np.stack(
        [results[bi]["dist"].reshape(N, N) for bi in range(B)], axis=-1
    )
    vh = (
        np.stack([results[bi]["vh"].reshape(3, N, N) for bi in range(B)], axis=0)
        .transpose(2, 3, 0, 1)
    )
    pm = np.arange(N)[None, :] < natoms[:, None]  # [B, N]
    mask = pm.T[:, None, :] & pm.T[None, :, :]  # [N, N, B]
    return feats, mask, pm, dist, vh


def kernel(pos, natoms, atomic_numbers, src_emb, dst_emb, rbf_w, rbf_b):
    in_maps, natoms_np = _make_in_maps(
        pos, natoms, atomic_numbers, src_emb, dst_emb, rbf_w, rbf_b
    )
    nc = _get_nc()
    res = run_bass_kernel_spmd(nc, in_maps, core_ids=list(range(B)))
    return _assemble(res.results, natoms_np)


# revision 23
# speedup vs baseline: 1.0392x; 1.0392x over previous
"""Trainium2 Bass kernel for nn_FeatureBuilder (pairwise RBF feature builder).

Strategy: data-parallel over the batch (graph) axis -- each of the 8
NeuronCores processes one padded graph [N=256 atoms]. Embedding tables /
RBF weights are replicated. Inside a core everything runs in an
[e(partition), pair(free)] layout.

Per unit of 4 atom-rows (1024 pairs):
  - the gaussian exponent arg  coeff*(d - c_k)^2  is expanded to
    coeff*d^2 - 2*coeff*c_k*d  (+ coeff*c_k^2 via the ScalarE bias) and
    built by a K=8 float32r matmul whose rows are exact 12-bit hi/lo
    (Dekker) splits of (d^2, d) x (coeff, -2*coeff*c_k) -- full fp32
    accuracy at the PE's 1-cycle/row float32r rate, landing directly in
    the [gaussian, pair] layout (no transposes, no broadcasts).
  - ONE ScalarE Exp (bias = coeff*c_k^2 per partition) -> g.
  - K=66 float32r matmul: rows 0-49 rbf_w/sqrt3 (stationary), rows
    50-65 a 16-row block of src-embedding rows (one slab DMA per 4
    units), selected by constant indicator rows preloaded in 4 rotating
    g tiles.
  - one DVE tensor_tensor adds the dst-embedding table during the
    mandatory PSUM->SBUF pass.
  - feats DMA'd in contiguous 1MB blocks (one DMA per 2 units).
"""

import math

import numpy as np

import concourse.bacc as bacc
import concourse.mybir as mybir
import concourse.tile as tile
from concourse.bass_utils import run_bass_kernel_spmd

# ---- problem constants (hardcoded per spec) ----
B = 8          # graphs == cores
N = 256        # padded atoms per graph (nmax)
P = 128        # partitions
NT = N // P    # i-tiles per graph
E = 128        # embed dim
G = 50         # gaussians
NELEM = 100
RADIUS = 12.0
SQRT3 = math.sqrt(3.0)

UNITS = N // 4          # 64 units of 4 atom-rows (1024 pairs each)
CHUNK_I = 32            # i-rows per sd chunk
UPC = 8                 # units per chunk
NCHUNK = N // CHUNK_I   # 8 chunks
NBLK = UNITS // 2       # feats DMA blocks (2 units each)

FP32 = mybir.dt.float32
FP32R = mybir.dt.float32r
AF = mybir.ActivationFunctionType
OP = mybir.AluOpType


def _split12(x):
    """Exact split of f32 array into hi (top 12 significand bits) + lo."""
    x = np.asarray(x, dtype=np.float32)
    hi = (x.view(np.uint32) & np.uint32(0xFFFFF000)).view(np.float32)
    lo = (x - hi).astype(np.float32)
    return hi, lo


def _rbf_consts():
    # match reference: float32 centers, coeff from f32 spacing
    centers = np.linspace(0.0, RADIUS, G, dtype=np.float32).astype(np.float64)
    coeff = -0.5 / float(np.float32(centers[1] - centers[0])) ** 2
    return centers, coeff


def _build():
    nc = bacc.Bacc("TRN2", target_bir_lowering=False, debug=False)

    # inputs (per-core shard + replicated tables)
    pos_pt = nc.dram_tensor("pos_pt", [NT, P, 3], FP32, kind="ExternalInput")
    pos_r = [
        nc.dram_tensor(f"pos_r{c}", [1, N], FP32, kind="ExternalInput")
        for c in range(3)
    ]
    anum_row = nc.dram_tensor("anum_row", [1, N], FP32, kind="ExternalInput")
    w_s = nc.dram_tensor("w_s", [G, E], FP32R, kind="ExternalInput")
    semb_s = nc.dram_tensor("semb_s", [NELEM, E], FP32, kind="ExternalInput")
    brow = nc.dram_tensor("brow", [1, E], FP32, kind="ExternalInput")
    demb_s = nc.dram_tensor("demb_s", [NELEM, E], FP32, kind="ExternalInput")
    qsplit = nc.dram_tensor("qsplit", [8, G], FP32R, kind="ExternalInput")
    qcol = nc.dram_tensor("qcol", [G, 1], FP32, kind="ExternalInput")
    iota_col = nc.dram_tensor("iota_col", [NELEM, 1], FP32, kind="ExternalInput")
    indv = nc.dram_tensor("indv", [64, 1024], FP32R, kind="ExternalInput")

    # outputs
    feats_o = nc.dram_tensor("feats", [NBLK, E, 2048], FP32, kind="ExternalOutput")
    dist_o = nc.dram_tensor("dist", [NT, P, N], FP32, kind="ExternalOutput")
    vh_o = nc.dram_tensor("vh", [3, NT, P, N], FP32, kind="ExternalOutput")

    with tile.TileContext(nc) as tc:
        with (
            tc.tile_pool(name="const", bufs=1) as cpool,
            tc.tile_pool(name="dwork", bufs=2) as dpool,
            tc.tile_pool(name="sd", bufs=2) as sdpool,
            tc.tile_pool(name="osb", bufs=3) as opool,
            tc.tile_pool(name="pf", bufs=2, space="PSUM") as pfpool,
            tc.tile_pool(name="pbc", bufs=2, space="PSUM") as pbcpool,
        ):
            # ---- load constants ----
            semb_sb = cpool.tile([NELEM, E], FP32)
            nc.sync.dma_start(semb_sb[:], semb_s[:])
            brow_sb = cpool.tile([1, E], FP32)
            nc.sync.dma_start(brow_sb[:], brow[:])
            demb_sb = cpool.tile([NELEM, E], FP32)
            nc.sync.dma_start(demb_sb[:], demb_s[:])
            # lives at partitions 96..103 so the K=8 exponent matmuls run in
            # PE row-group 3, concurrent with the K=66 feature matmuls
            qsplit_sb = cpool.tile([104, G], FP32R)
            nc.sync.dma_start(qsplit_sb[96:104, :], qsplit[:])
            qcol_sb = cpool.tile([G, 1], FP32)
            nc.sync.dma_start(qcol_sb[:], qcol[:])
            iota_sb = cpool.tile([NELEM, 1], FP32)
            nc.sync.dma_start(iota_sb[:], iota_col[:])
            anum_sb = cpool.tile([1, N], FP32)
            nc.sync.dma_start(anum_sb[:], anum_row[:])
            posr_sb = []
            for c in range(3):
                t_ = cpool.tile([1, N], FP32, tag=f"posr{c}")
                nc.sync.dma_start(t_[:], pos_r[c][:])
                posr_sb.append(t_)
            pt_sb = []
            for t in range(NT):
                t_ = cpool.tile([P, 3], FP32, tag=f"pt{t}")
                nc.sync.dma_start(t_[:], pos_pt[t])
                pt_sb.append(t_)

            ones100 = cpool.tile([1, NELEM], FP32)
            nc.gpsimd.memset(ones100[:], 1.0)
            ones128 = cpool.tile([1, P], FP32)
            nc.gpsimd.memset(ones128[:], 1.0)

            # ---- one-hot H^T [elem, atom] ----
            ps_an = pfpool.tile([NELEM, N], FP32, tag="pf")
            nc.tensor.matmul(ps_an[:], ones100[:], anum_sb[:])  # bcast anum rows
            Ht = cpool.tile([NELEM, N], FP32)
            nc.vector.tensor_scalar(Ht[:], ps_an[:], iota_sb[:], None, OP.is_equal)

            # src_pb[t][i_loc, e] = (src_emb[anum_i, e] + rbf_b[e]) / sqrt3
            src_pb = []
            for t in range(NT):
                ps_src = pfpool.tile([P, E], FP32, tag="pf")
                nc.tensor.matmul(
                    ps_src[:], Ht[:, t * P : (t + 1) * P], semb_sb[:],
                    start=True, stop=False,
                )
                nc.tensor.matmul(
                    ps_src[:], ones128[:], brow_sb[:], start=False, stop=True
                )
                t_ = cpool.tile([P, E], FP32, tag=f"srcpb{t}")
                nc.scalar.copy(t_[:], ps_src[:])
                src_pb.append(t_)

            # dstT4[e, 4*N] = dst_emb[anum_j, e]/sqrt3, repeated 4x along free
            ps_dst = pfpool.tile([E, N], FP32, tag="pf")
            nc.tensor.matmul(ps_dst[:], demb_sb[:], Ht[:])
            dstT4 = cpool.tile([E, 4 * N], FP32)
            for r in range(4):
                nc.scalar.copy(dstT4[:, r * N : (r + 1) * N], ps_dst[:])

            # ---- pos rows broadcast across partitions: pxb[p, j] = x_j ----
            pb_sb = []
            for c in range(3):
                ps_pb = pfpool.tile([P, N], FP32, tag="pf")
                nc.tensor.matmul(ps_pb[:], ones128[:], posr_sb[c][:])
                t_ = cpool.tile([P, N], FP32, tag=f"pb{c}")
                nc.scalar.copy(t_[:], ps_pb[:])
                pb_sb.append(t_)

            # ---- double-buffered lhsT for the feature matmul ----
            # rows 0..49 = rbf_w/sqrt3 (constant), rows 50..65 = a 16-row
            # block of src embedding rows (DMA'd once per 4 units)
            w66 = []
            for r in range(2):
                t_ = cpool.tile([G + 16, E], FP32R, tag=f"w66_{r}")
                nc.sync.dma_start(t_[0:G, :], w_s[:])
                w66.append(t_)

            # g tiles: rows 0..49 written by ACT each unit; rows 50..65 are
            # the constant indicator pattern for this unit's slab offset
            g66 = []
            for m in range(4):
                t_ = cpool.tile([G + 16, 4 * N], FP32R, tag=f"g66_{m}")
                nc.sync.dma_start(t_[G : G + 16, :], indv[16 * m : 16 * (m + 1), :])
                g66.append(t_)

            # ---- phase A: distances, vec_hat, hi/lo splits for both i-tiles ----
            shilo = []  # per t: (s_hi, s_lo, d_hi, d_lo)
            for t in range(NT):
                dvec = []
                for c in range(3):
                    dc = dpool.tile([P, N], FP32, tag=f"d{c}")
                    # vec[i, j] = pos[j] - pos[i]
                    nc.vector.tensor_scalar(
                        dc[:], pb_sb[c][:], pt_sb[t][:, c : c + 1], None, OP.subtract
                    )
                    dvec.append(dc)
                s_t = dpool.tile([P, N], FP32, tag="s")
                nc.vector.tensor_mul(s_t[:], dvec[0][:], dvec[0][:])
                tmp = dpool.tile([P, N], FP32, tag="tmp")
                nc.vector.tensor_mul(tmp[:], dvec[1][:], dvec[1][:])
                nc.vector.tensor_add(s_t[:], s_t[:], tmp[:])
                tmp2 = dpool.tile([P, N], FP32, tag="tmp2")
                nc.vector.tensor_mul(tmp2[:], dvec[2][:], dvec[2][:])
                nc.vector.tensor_add(s_t[:], s_t[:], tmp2[:])

                d_t = dpool.tile([P, N], FP32, tag="d")
                nc.scalar.sqrt(d_t[:], s_t[:])
                nc.sync.dma_start(dist_o[t], d_t[:])

                clamp = dpool.tile([P, N], FP32, tag="clamp")
                nc.vector.tensor_scalar(clamp[:], d_t[:], 1e-12, None, OP.max)
                inv = dpool.tile([P, N], FP32, tag="inv")
                nc.vector.reciprocal(inv[:], clamp[:])
                for c in range(3):
                    vh_t = dpool.tile([P, N], FP32, tag=f"vh{c}")
                    nc.vector.tensor_mul(vh_t[:], dvec[c][:], inv[:])
                    nc.sync.dma_start(vh_o[c, t], vh_t[:])

                # exact 12-bit hi/lo splits of s (=d^2) and d
                s_hi = dpool.tile([P, N], FP32, tag="s_hi")
                nc.vector.tensor_scalar(
                    s_hi[:].bitcast(mybir.dt.int32), s_t[:].bitcast(mybir.dt.int32),
                    -4096, None, OP.bitwise_and,
                )
                s_lo = dpool.tile([P, N], FP32, tag="s_lo")
                nc.vector.tensor_sub(s_lo[:], s_t[:], s_hi[:])
                d_hi = dpool.tile([P, N], FP32, tag="d_hi")
                nc.vector.tensor_scalar(
                    d_hi[:].bitcast(mybir.dt.int32), d_t[:].bitcast(mybir.dt.int32),
                    -4096, None, OP.bitwise_and,
                )
                d_lo = dpool.tile([P, N], FP32, tag="d_lo")
                nc.vector.tensor_sub(d_lo[:], d_t[:], d_hi[:])
                shilo.append((s_hi, s_lo, d_hi, d_lo))

            # ---- phase B: the RBF/feature unit loop, chunked over i ----
            for ch in range(NCHUNK):
                t = ch // (NCHUNK // NT)
                i0 = (ch % (NCHUNK // NT)) * CHUNK_I  # local i offset of chunk
                s_hi, s_lo, d_hi, d_lo = shilo[t]
                # sd8 rows 96..103: sh, sl, sh, sl, dh, dl, dh, dl
                sd8 = sdpool.tile([104, CHUNK_I * N], FP32R, tag="sd")
                for r, (eng, src_t) in enumerate(
                    (
                        (nc.sync, s_hi), (nc.sync, s_lo),
                        (nc.scalar, s_hi), (nc.scalar, s_lo),
                        (nc.sync, d_hi), (nc.sync, d_lo),
                        (nc.scalar, d_hi), (nc.scalar, d_lo),
                    )
                ):
                    row = 96 + r
                    eng.dma_start(
                        sd8[row : row + 1, :],
                        src_t[i0 : i0 + CHUNK_I, :].bitcast(FP32R),
                    )

                for v in range(UPC):  # 8 units per chunk
                    u = ch * UPC + v
                    m = v % 4

                    if m == 0:
                        # 16-row src slab for the next 4 units
                        wbuf = w66[(u // 4) % 2]
                        nc.sync.dma_start(
                            wbuf[G : G + 16, :],
                            src_pb[t][
                                i0 + 16 * (v // 4) : i0 + 16 * (v // 4) + 16, :
                            ].bitcast(FP32R),
                        )

                    ps_bc = pbcpool.tile([G, 4 * N], FP32, tag="pbc")
                    for q in range(2):
                        nc.tensor.matmul(
                            ps_bc[:, q * 512 : (q + 1) * 512],
                            qsplit_sb[96:104, :],
                            sd8[
                                96:104,
                                4 * v * N + q * 512 : 4 * v * N + (q + 1) * 512,
                            ],
                            tile_position=(96, 0),
                        )
                    gbuf = g66[m]
                    nc.scalar.activation(
                        gbuf[0:G, :], ps_bc[:], AF.Exp, bias=qcol_sb[:], scale=1.0
                    )
                    ps_f = pfpool.tile([E, 4 * N], FP32, tag="pf")
                    for q in range(2):
                        nc.tensor.matmul(
                            ps_f[:, q * 512 : (q + 1) * 512],
                            wbuf[:],
                            gbuf[:, q * 512 : (q + 1) * 512],
                        )
                    if u % 2 == 0:
                        o_sb = opool.tile([E, 2 * 4 * N], FP32, tag="osb")
                    nc.vector.tensor_add(
                        o_sb[:, (u % 2) * 1024 : (u % 2) * 1024 + 1024],
                        ps_f[:],
                        dstT4[:],
                    )
                    if u % 2 == 1:
                        nc.sync.dma_start(feats_o[u // 2], o_sb[:])

    nc.compile()
    return nc


_NC = None


def _get_nc():
    global _NC
    if _NC is None:
        _NC = _build()
    return _NC


def _make_in_maps(pos, natoms, atomic_numbers, src_emb, dst_emb, rbf_w, rbf_b):
    natoms = np.asarray(natoms).astype(np.int64)
    pos = np.asarray(pos, dtype=np.float32)
    anum = np.asarray(atomic_numbers).astype(np.int64)
    src = np.asarray(src_emb, dtype=np.float64)
    dst = np.asarray(dst_emb, dtype=np.float64)
    w = np.asarray(rbf_w, dtype=np.float64)
    bvec = np.asarray(rbf_b, dtype=np.float64)

    offs = np.concatenate([[0], np.cumsum(natoms)]).astype(np.int64)
    pp = np.zeros((B, N, 3), np.float32)
    pa = np.zeros((B, N), np.int64)
    for bi in range(B):
        na = int(natoms[bi])
        pp[bi, :na] = pos[offs[bi] : offs[bi] + na]
        pa[bi, :na] = anum[offs[bi] : offs[bi] + na]

    centers, coeff = _rbf_consts()
    ch, cl = _split12(np.full(G, coeff))
    q1h, q1l = _split12(-2.0 * coeff * centers)
    qsplit = np.stack([ch, ch, cl, cl, q1h, q1h, q1l, q1l], axis=0)
    qcol = (coeff * centers**2).astype(np.float32).reshape(G, 1)

    # indicator variants: variant m (unit u with u%4==m) activates slab rows
    # 4m..4m+3, one per atom-row of the unit
    indv = np.zeros((4, 16, 4 * N), np.float32)
    for m in range(4):
        for k in range(4):
            indv[m, 4 * m + k, k * N : (k + 1) * N] = 1.0
    indv = indv.reshape(64, 4 * N)

    shared = {
        "w_s": np.ascontiguousarray((w / SQRT3).astype(np.float32)),
        "semb_s": np.ascontiguousarray((src / SQRT3).astype(np.float32)),
        "brow": np.ascontiguousarray((bvec[None, :] / SQRT3).astype(np.float32)),
        "demb_s": np.ascontiguousarray((dst / SQRT3).astype(np.float32)),
        "qsplit": np.ascontiguousarray(qsplit.astype(np.float32)),
        "qcol": np.ascontiguousarray(qcol),
        "iota_col": np.arange(NELEM, dtype=np.float32).reshape(NELEM, 1),
        "indv": np.ascontiguousarray(indv),
    }
    in_maps = []
    for bi in range(B):
        m = dict(shared)
        m["pos_pt"] = np.ascontiguousarray(pp[bi].reshape(NT, P, 3))
        for c in range(3):
            m[f"pos_r{c}"] = np.ascontiguousarray(pp[bi][:, c].reshape(1, N))
        m["anum_row"] = np.ascontiguousarray(pa[bi].astype(np.float32).reshape(1, N))
        in_maps.append(m)
    return in_maps, natoms


def _assemble(results, natoms):
    # feats dev layout: [NBLK, E, 2048] with block b = atom rows 8b..8b+7
    feats = (
        np.stack([results[bi]["feats"] for bi in range(B)], axis=0)
        .reshape(B, NBLK, E, 8, N)
        .transpose(1, 3, 4, 0, 2)
        .reshape(N, N, B, E)
    )
    dist = np.stack(
        [results[bi]["dist"].reshape(N, N) for bi in range(B)], axis=-1
    )
    vh = (
        np.stack([results[bi]["vh"].reshape(3, N, N) for bi in range(B)], axis=0)
        .transpose(2, 3, 0, 1)
    )
    pm = np.arange(N)[None, :] < natoms[:, None]  # [B, N]
    mask = pm.T[:, None, :] & pm.T[None, :, :]  # [N, N, B]
    return feats, mask, pm, dist, vh


def kernel(pos, natoms, atomic_numbers, src_emb, dst_emb, rbf_w, rbf_b):
    in_maps, natoms_np = _make_in_maps(
        pos, natoms, atomic_numbers, src_emb, dst_emb, rbf_w, rbf_b
    )
    nc = _get_nc()
    res = run_bass_kernel_spmd(nc, in_maps, core_ids=list(range(B)))
    return _assemble(res.results, natoms_np)


# revision 25
# speedup vs baseline: 1.1028x; 1.0612x over previous
"""Trainium2 Bass kernel for nn_FeatureBuilder (pairwise RBF feature builder).

Strategy: data-parallel over the batch (graph) axis -- each of the 8
NeuronCores processes one padded graph [N=256 atoms]. Embedding tables /
RBF weights are replicated. Inside a core everything runs in an
[e(partition), pair(free)] layout.

Per unit of 4 atom-rows (1024 pairs):
  - the gaussian exponent arg  coeff*(d - c_k)^2  is expanded to
    coeff*d^2 - 2*coeff*c_k*d  (+ coeff*c_k^2 via the ScalarE bias) and
    built by a K=8 float32r matmul whose rows are exact 12-bit hi/lo
    (Dekker) splits of (d^2, d) x (coeff, -2*coeff*c_k) -- full fp32
    accuracy at the PE's 1-cycle/row float32r rate, landing directly in
    the [gaussian, pair] layout (no transposes, no broadcasts).
  - ONE ScalarE Exp (bias = coeff*c_k^2 per partition) -> g.
  - K=66 float32r matmul: rows 0-49 rbf_w/sqrt3 (stationary), rows
    50-65 a 16-row block of src-embedding rows (one slab DMA per 4
    units), selected by constant indicator rows preloaded in 4 rotating
    g tiles.
  - one DVE tensor_tensor adds the dst-embedding table during the
    mandatory PSUM->SBUF pass.
  - feats DMA'd in contiguous 1MB blocks (one DMA per 2 units).
"""

import math

import numpy as np

import concourse.bacc as bacc
import concourse.mybir as mybir
import concourse.tile as tile
from concourse.bass_utils import run_bass_kernel_spmd

# ---- problem constants (hardcoded per spec) ----
B = 8          # graphs == cores
N = 256        # padded atoms per graph (nmax)
P = 128        # partitions
NT = N // P    # i-tiles per graph
E = 128        # embed dim
G = 50         # gaussians
NELEM = 100
RADIUS = 12.0
SQRT3 = math.sqrt(3.0)

UNITS = N // 4          # 64 units of 4 atom-rows (1024 pairs each)
CHUNK_I = 32            # i-rows per sd chunk
UPC = 8                 # units per chunk
NCHUNK = N // CHUNK_I   # 8 chunks
NBLK = UNITS // 2       # feats DMA blocks (2 units each)

FP32 = mybir.dt.float32
FP32R = mybir.dt.float32r
AF = mybir.ActivationFunctionType
OP = mybir.AluOpType


def _split12(x):
    """Exact split of f32 array into hi (top 12 significand bits) + lo."""
    x = np.asarray(x, dtype=np.float32)
    hi = (x.view(np.uint32) & np.uint32(0xFFFFF000)).view(np.float32)
    lo = (x - hi).astype(np.float32)
    return hi, lo


def _rbf_consts():
    # match reference: float32 centers, coeff from f32 spacing
    centers = np.linspace(0.0, RADIUS, G, dtype=np.float32).astype(np.float64)
    coeff = -0.5 / float(np.float32(centers[1] - centers[0])) ** 2
    return centers, coeff


def _build():
    nc = bacc.Bacc("TRN2", target_bir_lowering=False, debug=False)

    # inputs (per-core shard + replicated tables)
    pos_pt = nc.dram_tensor("pos_pt", [NT, P, 3], FP32, kind="ExternalInput")
    pos_r = [
        nc.dram_tensor(f"pos_r{c}", [1, N], FP32, kind="ExternalInput")
        for c in range(3)
    ]
    anum_row = nc.dram_tensor("anum_row", [1, N], FP32, kind="ExternalInput")
    w_s = nc.dram_tensor("w_s", [G, E], FP32R, kind="ExternalInput")
    semb_s = nc.dram_tensor("semb_s", [NELEM, E], FP32, kind="ExternalInput")
    brow = nc.dram_tensor("brow", [1, E], FP32, kind="ExternalInput")
    demb_s = nc.dram_tensor("demb_s", [NELEM, E], FP32, kind="ExternalInput")
    qsplit = nc.dram_tensor("qsplit", [8, G], FP32R, kind="ExternalInput")
    qcol = nc.dram_tensor("qcol", [G, 1], FP32, kind="ExternalInput")
    iota_col = nc.dram_tensor("iota_col", [NELEM, 1], FP32, kind="ExternalInput")
    indv = nc.dram_tensor("indv", [64, 1024], FP32R, kind="ExternalInput")

    # outputs
    feats_o = nc.dram_tensor("feats", [NBLK, E, 2048], FP32, kind="ExternalOutput")
    dist_o = nc.dram_tensor("dist", [NT, P, N], FP32, kind="ExternalOutput")
    vh_o = nc.dram_tensor("vh", [3, NT, P, N], FP32, kind="ExternalOutput")

    with tile.TileContext(nc) as tc:
        with (
            tc.tile_pool(name="const", bufs=1) as cpool,
            tc.tile_pool(name="dwork", bufs=2) as dpool,
            tc.tile_pool(name="sd", bufs=3) as sdpool,
            tc.tile_pool(name="osb", bufs=3) as opool,
            tc.tile_pool(name="pf", bufs=2, space="PSUM") as pfpool,
            tc.tile_pool(name="pbc", bufs=2, space="PSUM") as pbcpool,
        ):
            # ---- load constants ----
            semb_sb = cpool.tile([NELEM, E], FP32)
            nc.sync.dma_start(semb_sb[:], semb_s[:])
            brow_sb = cpool.tile([1, E], FP32)
            nc.sync.dma_start(brow_sb[:], brow[:])
            demb_sb = cpool.tile([NELEM, E], FP32)
            nc.sync.dma_start(demb_sb[:], demb_s[:])
            # lives at partitions 96..103 so the K=8 exponent matmuls run in
            # PE row-group 3, concurrent with the K=66 feature matmuls
            qsplit_sb = cpool.tile([104, G], FP32R)
            nc.sync.dma_start(qsplit_sb[96:104, :], qsplit[:])
            qcol_sb = cpool.tile([G, 1], FP32)
            nc.sync.dma_start(qcol_sb[:], qcol[:])
            iota_sb = cpool.tile([NELEM, 1], FP32)
            nc.sync.dma_start(iota_sb[:], iota_col[:])
            anum_sb = cpool.tile([1, N], FP32)
            nc.sync.dma_start(anum_sb[:], anum_row[:])
            posr_sb = []
            for c in range(3):
                t_ = cpool.tile([1, N], FP32, tag=f"posr{c}")
                nc.sync.dma_start(t_[:], pos_r[c][:])
                posr_sb.append(t_)
            pt_sb = []
            for t in range(NT):
                t_ = cpool.tile([P, 3], FP32, tag=f"pt{t}")
                nc.sync.dma_start(t_[:], pos_pt[t])
                pt_sb.append(t_)

            ones100 = cpool.tile([1, NELEM], FP32)
            nc.gpsimd.memset(ones100[:], 1.0)
            ones128 = cpool.tile([1, P], FP32)
            nc.gpsimd.memset(ones128[:], 1.0)

            # ---- one-hot H^T [elem, atom] ----
            ps_an = pfpool.tile([NELEM, N], FP32, tag="pf")
            nc.tensor.matmul(ps_an[:], ones100[:], anum_sb[:])  # bcast anum rows
            Ht = cpool.tile([NELEM, N], FP32)
            nc.vector.tensor_scalar(Ht[:], ps_an[:], iota_sb[:], None, OP.is_equal)

            # src_pb[t][i_loc, e] = (src_emb[anum_i, e] + rbf_b[e]) / sqrt3
            src_pb = []
            for t in range(NT):
                ps_src = pfpool.tile([P, E], FP32, tag="pf")
                nc.tensor.matmul(
                    ps_src[:], Ht[:, t * P : (t + 1) * P], semb_sb[:],
                    start=True, stop=False,
                )
                nc.tensor.matmul(
                    ps_src[:], ones128[:], brow_sb[:], start=False, stop=True
                )
                t_ = cpool.tile([P, E], FP32, tag=f"srcpb{t}")
                nc.scalar.copy(t_[:], ps_src[:])
                src_pb.append(t_)

            # dstT4[e, 4*N] = dst_emb[anum_j, e]/sqrt3, repeated 4x along free
            ps_dst = pfpool.tile([E, N], FP32, tag="pf")
            nc.tensor.matmul(ps_dst[:], demb_sb[:], Ht[:])
            dstT4 = cpool.tile([E, 4 * N], FP32)
            for r in range(4):
                nc.scalar.copy(dstT4[:, r * N : (r + 1) * N], ps_dst[:])

            # ---- pos rows broadcast across partitions: pxb[p, j] = x_j ----
            pb_sb = []
            for c in range(3):
                ps_pb = pfpool.tile([P, N], FP32, tag="pf")
                nc.tensor.matmul(ps_pb[:], ones128[:], posr_sb[c][:])
                t_ = cpool.tile([P, N], FP32, tag=f"pb{c}")
                nc.scalar.copy(t_[:], ps_pb[:])
                pb_sb.append(t_)

            # ---- double-buffered lhsT for the feature matmul ----
            # rows 0..49 = rbf_w/sqrt3 (constant), rows 50..65 = a 16-row
            # block of src embedding rows (DMA'd once per 4 units)
            w66 = []
            for r in range(2):
                t_ = cpool.tile([G + 16, E], FP32R, tag=f"w66_{r}")
                nc.sync.dma_start(t_[0:G, :], w_s[:])
                w66.append(t_)

            # g tiles: rows 0..49 written by ACT each unit; rows 50..65 are
            # the constant indicator pattern for this unit's slab offset
            g66 = []
            for m in range(4):
                t_ = cpool.tile([G + 16, 4 * N], FP32R, tag=f"g66_{m}")
                nc.sync.dma_start(t_[G : G + 16, :], indv[16 * m : 16 * (m + 1), :])
                g66.append(t_)

            # ---- phase A: distances, vec_hat, hi/lo splits for both i-tiles ----
            shilo = []  # per t: (s_hi, s_lo, d_hi, d_lo)
            for t in range(NT):
                dvec = []
                for c in range(3):
                    dc = dpool.tile([P, N], FP32, tag=f"d{c}")
                    # vec[i, j] = pos[j] - pos[i]
                    nc.vector.tensor_scalar(
                        dc[:], pb_sb[c][:], pt_sb[t][:, c : c + 1], None, OP.subtract
                    )
                    dvec.append(dc)
                s_t = dpool.tile([P, N], FP32, tag="s")
                nc.vector.tensor_mul(s_t[:], dvec[0][:], dvec[0][:])
                tmp = dpool.tile([P, N], FP32, tag="tmp")
                nc.vector.tensor_mul(tmp[:], dvec[1][:], dvec[1][:])
                nc.vector.tensor_add(s_t[:], s_t[:], tmp[:])
                tmp2 = dpool.tile([P, N], FP32, tag="tmp2")
                nc.vector.tensor_mul(tmp2[:], dvec[2][:], dvec[2][:])
                nc.vector.tensor_add(s_t[:], s_t[:], tmp2[:])

                d_t = dpool.tile([P, N], FP32, tag="d")
                nc.scalar.sqrt(d_t[:], s_t[:])
                nc.sync.dma_start(dist_o[t], d_t[:])

                clamp = dpool.tile([P, N], FP32, tag="clamp")
                nc.vector.tensor_scalar(clamp[:], d_t[:], 1e-12, None, OP.max)
                inv = dpool.tile([P, N], FP32, tag="inv")
                nc.vector.reciprocal(inv[:], clamp[:])
                for c in range(3):
                    vh_t = dpool.tile([P, N], FP32, tag=f"vh{c}")
                    nc.vector.tensor_mul(vh_t[:], dvec[c][:], inv[:])
                    nc.sync.dma_start(vh_o[c, t], vh_t[:])

                # exact 12-bit hi/lo splits of s (=d^2) and d
                s_hi = dpool.tile([P, N], FP32, tag="s_hi")
                nc.vector.tensor_scalar(
                    s_hi[:].bitcast(mybir.dt.int32), s_t[:].bitcast(mybir.dt.int32),
                    -4096, None, OP.bitwise_and,
                )
                s_lo = dpool.tile([P, N], FP32, tag="s_lo")
                nc.vector.tensor_sub(s_lo[:], s_t[:], s_hi[:])
                d_hi = dpool.tile([P, N], FP32, tag="d_hi")
                nc.vector.tensor_scalar(
                    d_hi[:].bitcast(mybir.dt.int32), d_t[:].bitcast(mybir.dt.int32),
                    -4096, None, OP.bitwise_and,
                )
                d_lo = dpool.tile([P, N], FP32, tag="d_lo")
                nc.vector.tensor_sub(d_lo[:], d_t[:], d_hi[:])
                shilo.append((s_hi, s_lo, d_hi, d_lo))

            # ---- phase B: the RBF/feature unit loop, chunked over i ----
            # sd8 packs are emitted one chunk ahead so their (slow,
            # single-partition-fanin) transfers overlap the previous
            # chunk's compute instead of stalling the boundary.
            def emit_pack(ch):
                t = ch // (NCHUNK // NT)
                i0 = (ch % (NCHUNK // NT)) * CHUNK_I
                s_hi, s_lo, d_hi, d_lo = shilo[t]
                # sd8 rows 96..103: sh, sl, sh, sl, dh, dl, dh, dl
                sd8 = sdpool.tile([104, CHUNK_I * N], FP32R, tag="sd")
                for r, (eng, src_t) in enumerate(
                    (
                        (nc.sync, s_hi), (nc.sync, s_lo),
                        (nc.scalar, s_hi), (nc.scalar, s_lo),
                        (nc.sync, d_hi), (nc.sync, d_lo),
                        (nc.scalar, d_hi), (nc.scalar, d_lo),
                    )
                ):
                    row = 96 + r
                    eng.dma_start(
                        sd8[row : row + 1, :],
                        src_t[i0 : i0 + CHUNK_I, :].bitcast(FP32R),
                    )
                return sd8

            sd_tiles = {0: emit_pack(0)}
            for ch in range(NCHUNK):
                if ch + 1 < NCHUNK:
                    sd_tiles[ch + 1] = emit_pack(ch + 1)
                t = ch // (NCHUNK // NT)
                i0 = (ch % (NCHUNK // NT)) * CHUNK_I  # local i offset of chunk
                sd8 = sd_tiles.pop(ch)

                for v in range(UPC):  # 8 units per chunk
                    u = ch * UPC + v
                    m = v % 4

                    if m == 0:
                        # 16-row src slab for the next 4 units
                        wbuf = w66[(u // 4) % 2]
                        nc.sync.dma_start(
                            wbuf[G : G + 16, :],
                            src_pb[t][
                                i0 + 16 * (v // 4) : i0 + 16 * (v // 4) + 16, :
                            ].bitcast(FP32R),
                        )

                    ps_bc = pbcpool.tile([G, 4 * N], FP32, tag="pbc")
                    for q in range(2):
                        nc.tensor.matmul(
                            ps_bc[:, q * 512 : (q + 1) * 512],
                            qsplit_sb[96:104, :],
                            sd8[
                                96:104,
                                4 * v * N + q * 512 : 4 * v * N + (q + 1) * 512,
                            ],
                            tile_position=(96, 0),
                        )
                    gbuf = g66[m]
                    nc.scalar.activation(
                        gbuf[0:G, :], ps_bc[:], AF.Exp, bias=qcol_sb[:], scale=1.0
                    )
                    ps_f = pfpool.tile([E, 4 * N], FP32, tag="pf")
                    for q in range(2):
                        nc.tensor.matmul(
                            ps_f[:, q * 512 : (q + 1) * 512],
                            wbuf[:],
                            gbuf[:, q * 512 : (q + 1) * 512],
                        )
                    if u % 2 == 0:
                        o_sb = opool.tile([E, 2 * 4 * N], FP32, tag="osb")
                    nc.vector.tensor_add(
                        o_sb[:, (u % 2) * 1024 : (u % 2) * 1024 + 1024],
                        ps_f[:],
                        dstT4[:],
                    )
                    if u % 2 == 1:
                        nc.sync.dma_start(feats_o[u // 2], o_sb[:])

    nc.compile()
    return nc


_NC = None


def _get_nc():
    global _NC
    if _NC is None:
        _NC = _build()
    return _NC


def _make_in_maps(pos, natoms, atomic_numbers, src_emb, dst_emb, rbf_w, rbf_b):
    natoms = np.asarray(natoms).astype(np.int64)
    pos = np.asarray(pos, dtype=np.float32)
    anum = np.asarray(atomic_numbers).astype(np.int64)
    src = np.asarray(src_emb, dtype=np.float64)
    dst = np.asarray(dst_emb, dtype=np.float64)
    w = np.asarray(rbf_w, dtype=np.float64)
    bvec = np.asarray(rbf_b, dtype=np.float64)

    offs = np.concatenate([[0], np.cumsum(natoms)]).astype(np.int64)
    pp = np.zeros((B, N, 3), np.float32)
    pa = np.zeros((B, N), np.int64)
    for bi in range(B):
        na = int(natoms[bi])
        pp[bi, :na] = pos[offs[bi] : offs[bi] + na]
        pa[bi, :na] = anum[offs[bi] : offs[bi] + na]

    centers, coeff = _rbf_consts()
    ch, cl = _split12(np.full(G, coeff))
    q1h, q1l = _split12(-2.0 * coeff * centers)
    qsplit = np.stack([ch, ch, cl, cl, q1h, q1h, q1l, q1l], axis=0)
    qcol = (coeff * centers**2).astype(np.float32).reshape(G, 1)

    # indicator variants: variant m (unit u with u%4==m) activates slab rows
    # 4m..4m+3, one per atom-row of the unit
    indv = np.zeros((4, 16, 4 * N), np.float32)
    for m in range(4):
        for k in range(4):
            indv[m, 4 * m + k, k * N : (k + 1) * N] = 1.0
    indv = indv.reshape(64, 4 * N)

    shared = {
        "w_s": np.ascontiguousarray((w / SQRT3).astype(np.float32)),
        "semb_s": np.ascontiguousarray((src / SQRT3).astype(np.float32)),
        "brow": np.ascontiguousarray((bvec[None, :] / SQRT3).astype(np.float32)),
        "demb_s": np.ascontiguousarray((dst / SQRT3).astype(np.float32)),
        "qsplit": np.ascontiguousarray(qsplit.astype(np.float32)),
        "qcol": np.ascontiguousarray(qcol),
        "iota_col": np.arange(NELEM, dtype=np.float32).reshape(NELEM, 1),
        "indv": np.ascontiguousarray(indv),
    }
    in_maps = []
    for bi in range(B):
        m = dict(shared)
        m["pos_pt"] = np.ascontiguousarray(pp[bi].reshape(NT, P, 3))
        for c in range(3):
            m[f"pos_r{c}"] = np.ascontiguousarray(pp[bi][:, c].reshape(1, N))
        m["anum_row"] = np.ascontiguousarray(pa[bi].astype(np.float32).reshape(1, N))
        in_maps.append(m)
    return in_maps, natoms


def _assemble(results, natoms):
    # feats dev layout: [NBLK, E, 2048] with block b = atom rows 8b..8b+7
    feats = (
        np.stack([results[bi]["feats"] for bi in range(B)], axis=0)
        .reshape(B, NBLK, E, 8, N)
        .transpose(1, 3, 4, 0, 2)
        .reshape(N, N, B, E)
    )
    dist = np.stack(
        [results[bi]["dist"].reshape(N, N) for bi in range(B)], axis=-1
    )
    vh = (
        np.stack([results[bi]["vh"].reshape(3, N, N) for bi in range(B)], axis=0)
        .transpose(2, 3, 0, 1)
    )
    pm = np.arange(N)[None, :] < natoms[:, None]  # [B, N]
    mask = pm.T[:, None, :] & pm.T[None, :, :]  # [N, N, B]
    return feats, mask, pm, dist, vh


def kernel(pos, natoms, atomic_numbers, src_emb, dst_emb, rbf_w, rbf_b):
    in_maps, natoms_np = _make_in_maps(
        pos, natoms, atomic_numbers, src_emb, dst_emb, rbf_w, rbf_b
    )
    nc = _get_nc()
    res = run_bass_kernel_spmd(nc, in_maps, core_ids=list(range(B)))
    return _assemble(res.results, natoms_np)
